# revision 9
# baseline (speedup 1.0000x reference)
"""Trainium2 Bass kernel for nn_BasicBlock (distance-transform conv BasicBlock).

Computes: relu(bn2(dt_conv2(relu(bn1(dt_conv1(x))))) + x)
where dt_conv is a 3x3 "distance transform conv":
    d[b,o,h,w] = sqrt(||p - c_o||^2),  p = 3x3 zero-padded patch (dim 576)

Strategy (8 NeuronCores, data-parallel over batch 32 -> 4 images/core):
- ||p||^2 - 2 p.c in ONE matmul accumulation group per pixel tile: SBUF
  partitions 0:64 hold x (weights = -2*centers), partitions 64:128 hold x^2
  (weights = 1.0). 9 shifted matmuls (3x3 offsets) accumulate in PSUM,
  K=128, M=64 out channels, N=392 (7 rows x 56), 8 row-groups per image.
- PE column-pairing: images (0,2) and (1,3) share one PSUM bank pair —
  image A accumulates into psum[0:64], image B into psum[64:128]; the two
  64-col PE groups run concurrently (~2x effective matmul throughput).
- PSUM tiles span 2 banks (2 row-groups); evictions are one ACT sqrt over
  [128, 2, 7, 56] with accum_out giving per-channel sum(d). sum(d^2) =
  sum(psum) + n*c2 via one DVE XYZ-reduce per bank pair.
- d stored fp16 (quantization ~0.5% of the BN sigma; keeps DVE ops in
  2-byte 4x mode). Glue relu(s1*d+t1) runs on DVE (2 tensor_scalar ops)
  instead of ACT so ACT only does sqrt evictions.
- Sync-BN: fold upper/lower halves, [64,2] AllGather across 8 cores +
  local rank-sum, x2 layers (exact batch stats; Taylor/local-stat
  shortcuts measured > 2e-2 rel err and were rejected).
- Final: a = s2*d+t2 (DVE ts, fp16), a += xres (DVE tt), out =
  max(a,0) (DVE ts, bf16) — ACT stays free so next rep's evictions are
  not FIFO-blocked behind the final phase.
- Cross-rep software pipelining: emission order interleaves rep r's
  barrier-2/final phases with rep r+1's input DMA + layer-1 matmuls, so
  the PE never idles during collectives (d tile double-buffered).

kernel(**inputs) takes FULL unsharded inputs, returns FULL output.
Self-contained: shapes/sharding hardcoded; no file reads.
"""
import numpy as np

from concourse import bacc, mybir, tile
from concourse.bass_utils import run_bass_kernel_spmd

f32 = mybir.dt.float32
bf16 = mybir.dt.bfloat16
fp16 = mybir.dt.float16
ADD = mybir.AluOpType.add
MULT = mybir.AluOpType.mult
SUB = mybir.AluOpType.subtract
MAX = mybir.AluOpType.max
AF = mybir.ActivationFunctionType
XYZ = mybir.AxisListType.XYZ

N_CORES = 8
B_LOCAL = 4            # images per core (32 / 8)
C = 64                 # channels (in == out)
HW = 56                # spatial
HP = HW + 2            # padded
RPG = 7                # rows per matmul group (N = 7*56 = 392)
NGRP = 8               # row-groups per image-pair slot (56 / 7)
NPAIR = 4              # bank pairs (2 groups) per slot
N_GLOBAL = 32 * HW * HW
BN_EPS = 1e-5


def _pb(b):
    """Partition base and pair-slot index for the 128-wide d layout."""
    return 64 * (b // 2), b % 2


def _bn_affine(nc, pool, gstats, zc2, gamma, beta, eps, name):
    """From [sum(d), sum(d^2)-n*c2] (dup both halves) -> scale s, shift t
    [128,1]. zc2 is a [128, 2] cst slice [zero, c2]."""
    P = 2 * C
    mued = pool.tile([P, 2], f32, tag=f"mued_{name}")
    nvar = pool.tile([P, 1], f32, tag=f"nvar_{name}")
    sd = pool.tile([P, 1], f32, tag=f"sd_{name}")
    inv = pool.tile([P, 1], f32, tag=f"inv_{name}")
    s = pool.tile([P, 1], f32, tag=f"s_{name}")
    st = pool.tile([P, 1], f32, tag=f"st_{name}")
    tt = pool.tile([P, 1], f32, tag=f"t_{name}")
    inv_n = 1.0 / float(N_GLOBAL)
    # [mu, E[d^2]] = gstats * 1/N + [0, c2] in one DVE op
    nc.vector.scalar_tensor_tensor(
        out=mued[:, :], in0=gstats[:, 0:2], scalar=inv_n, in1=zc2,
        op0=MULT, op1=ADD)
    mu, ed2 = mued[:, 0:1], mued[:, 1:2]
    # -var = mu*mu - E[d^2] in one STT; sqrt flips the sign via scale=-1
    nc.vector.scalar_tensor_tensor(
        out=nvar[:, :], in0=mu, scalar=mu, in1=ed2, op0=MULT, op1=SUB)
    nc.scalar.activation(out=sd[:, :], in_=nvar[:, :], func=AF.Sqrt,
                         bias=eps[:, 0:1], scale=-1.0)
    nc.vector.reciprocal(out=inv[:, :], in_=sd[:, :])
    nc.vector.tensor_tensor(out=s[:, :], in0=gamma, in1=inv[:, :], op=MULT)
    nc.vector.tensor_tensor(out=st[:, :], in0=mu, in1=s[:, :], op=MULT)
    nc.vector.tensor_tensor(out=tt[:, :], in0=beta, in1=st[:, :], op=SUB)
    return s, tt


def _stats_allreduce(nc, pool, dram, sumd, sumps, name):
    """Reduce [128, 8] stat columns, fold upper half into lower, AllGather
    [64,2,2] across 8 cores + local rank-sum, return [128,2] duplicated
    global sums."""
    red = pool.tile([2 * C, 2], f32, tag=f"red_{name}")
    gstats = pool.tile([2 * C, 2], f32, tag=f"gstats_{name}")
    nc.vector.tensor_reduce(out=red[:, 0:1], in_=sumd[:, :],
                            axis=mybir.AxisListType.X, op=ADD)
    nc.vector.tensor_reduce(out=red[:, 1:2], in_=sumps[:, :],
                            axis=mybir.AxisListType.X, op=ADD)
    # images (2,3) stats live on the upper partition half; DMA both halves
    # side by side into the collective input (DVE tensor ops can't mix
    # partition bases, DMA can) and fold during the rank-sum reduce.
    cc_in = dram.tile([C, 2, 2], f32, tag=f"ccin_{name}")
    nc.sync.dma_start(out=cc_in[:, :, :].rearrange("c s h -> h c s"),
                      in_=red[:, :])
    cc_out = dram.tile([N_CORES * C, 2, 2], f32, tag=f"ccout_{name}")
    gag = pool.tile([C, N_CORES, 2, 2], f32, tag=f"gag_{name}")
    nc.gpsimd.collective_compute(
        "AllGather", mybir.AluOpType.bypass,
        replica_groups=[list(range(N_CORES))],
        ins=[cc_in.opt()],
        outs=[cc_out.opt()],
    )
    nc.sync.dma_start(
        out=gag[:, :, :, :],
        in_=cc_out[:, :, :].rearrange("(r c) s h -> c r s h", r=N_CORES))
    nc.vector.tensor_reduce(out=gstats[0:C, 0:1], in_=gag[:, :, 0, :],
                            axis=mybir.AxisListType.XY, op=ADD)
    nc.vector.tensor_reduce(out=gstats[0:C, 1:2], in_=gag[:, :, 1, :],
                            axis=mybir.AxisListType.XY, op=ADD)
    nc.vector.tensor_copy(out=gstats[C:2 * C, :], in_=gstats[0:C, :])
    return gstats


def build(reps=1):
    nc = bacc.Bacc("TRN2", target_bir_lowering=False, debug=False,
                   num_devices=N_CORES)
    x_ext = nc.declare_dram_parameter("x", [B_LOCAL, C, HW, HW], bf16,
                                      isOutput=False)
    w1_ext = nc.declare_dram_parameter("w1", [2 * C, 9, C], bf16, isOutput=False)
    w2_ext = nc.declare_dram_parameter("w2", [2 * C, 9, C], bf16, isOutput=False)
    # packed [zero | c2a | zero | c2b | g1 | b1 | g2 | b2], dup on both halves
    cst_ext = nc.declare_dram_parameter("cst", [2 * C, 8], f32, isOutput=False)
    out_ext = nc.declare_dram_parameter("out", [B_LOCAL, C, HW, HW], bf16,
                                        isOutput=True)

    with tile.TileContext(nc) as tc:
        with (
            tc.tile_pool(name="big", bufs=1) as big,
            tc.tile_pool(name="dbuf", bufs=2) as dbuf,
            tc.tile_pool(name="small", bufs=1) as pool,
            tc.tile_pool(name="psum", bufs=4, space="PSUM") as psum,
            tc.tile_pool(name="dram", bufs=1, space="DRAM") as dram,
        ):
            w1 = pool.tile([2 * C, 9, C], bf16, tag="w1")
            w2 = pool.tile([2 * C, 9, C], bf16, tag="w2")
            cst = pool.tile([2 * C, 8], f32, tag="cst")
            g1, b1 = cst[:, 4:5], cst[:, 5:6]
            g2, b2 = cst[:, 6:7], cst[:, 7:8]
            eps = pool.tile([2 * C, 1], f32, tag="eps")
            nc.vector.memset(eps[:, :], BN_EPS)
            # constants via the gpsimd SWDGE ring (SP/ACT rings carry x)
            nc.gpsimd.dma_start(out=w1[:, :, :], in_=w1_ext[:, :, :])
            nc.gpsimd.dma_start(out=cst[:, :], in_=cst_ext[:, :])
            nc.gpsimd.dma_start(out=w2[:, :, :], in_=w2_ext[:, :, :])

            # long-lived tiles (one logical tile, reused every rep; subtile
            # deps order cross-rep WAR/RAW). d rotates through 2 buffers so
            # rep r+1's evictions don't wait on rep r's final-phase reads.
            xt = [big.tile([2 * C, HP, HP], bf16, tag=f"xt{b}",
                           name=f"xt{b}") for b in range(B_LOCAL)]
            yt = [big.tile([2 * C, HP, HP], bf16, tag=f"yt{b}",
                           name=f"yt{b}") for b in range(B_LOCAL)]
            xres = big.tile([2 * C, 2, HW, HW], bf16, tag="xres")
            afin = big.tile([2 * C, 2, HW, HW], fp16, tag="afin")
            dout = big.tile([2 * C, 2, HW, HW], bf16, tag="dout")
            sumd1 = pool.tile([2 * C, 2 * NPAIR], f32, tag="sumd1")
            sumps1 = pool.tile([2 * C, 2 * NPAIR], f32, tag="sumps1")
            sumd2 = pool.tile([2 * C, 2 * NPAIR], f32, tag="sumd2")
            sumps2 = pool.tile([2 * C, 2 * NPAIR], f32, tag="sumps2")

            # per-rep state carried between stage emitters
            st = {}

            def layer_unit(src, w, ci, d, sumd, sumps, p, i):
                """Matmuls + eviction + psum-reduce for bank pair p, slot i.
                src[b] is a [128, HP, HP] bf16 tile (x | x^2)."""
                ps = psum.tile([2 * C, 2, NGRP, 64], f32, tag="ps")
                for j in range(2):
                    r0 = (2 * p + j) * RPG
                    for k in range(9):
                        kh, kw = k // 3, k % 3
                        # col-tiled pair: images i and i+2 occupy disjoint
                        # 64-col PE groups and run concurrently (~2x)
                        nc.tensor.matmul(
                            ps[0:C, j, 0:RPG, 0:HW],
                            w[:, k, :],
                            src[i][:, r0 + kh:r0 + kh + RPG, kw:kw + HW],
                            start=(k == 0), stop=(k == 8),
                            tile_position=(0, 0),
                        )
                        nc.tensor.matmul(
                            ps[C:2 * C, j, 0:RPG, 0:HW],
                            w[:, k, :],
                            src[i + 2][:, r0 + kh:r0 + kh + RPG, kw:kw + HW],
                            start=(k == 0), stop=(k == 8),
                            tile_position=(0, 64),
                        )
                col = i * NPAIR + p
                # per-channel sum(psum) (-> sum(d^2) after +n*c2); emitted
                # first so it runs concurrently with the ACT eviction
                nc.vector.tensor_reduce(
                    out=sumps[:, col:col + 1],
                    in_=ps[:, :, 0:RPG, 0:HW],
                    axis=XYZ, op=ADD)
                # d = sqrt(psum + ||c||^2); accum_out gives per-channel sum(d)
                r0 = 2 * p * RPG
                ev = nc.scalar.activation(
                    out=d[:, i, r0:r0 + 2 * RPG, :].rearrange(
                        "c (a b) w -> c a b w", a=2),
                    in_=ps[:, :, 0:RPG, 0:HW],
                    func=AF.Sqrt, bias=cst[:, ci:ci + 1], scale=1.0,
                    accum_out=sumd[:, col:col + 1])
                return ev

            def emit_A(r):
                """Input DMA + squares into padded xt tiles."""
                if r == 0:
                    # zero the pad borders once (interior-only writes after
                    # this keep them zero). x tiles on DVE, y tiles on Pool.
                    for tiles, eng in ((xt, nc.vector), (yt, nc.gpsimd)):
                        for t in tiles:
                            eng.memset(t[:, 0:1, :], 0.0)
                            eng.memset(t[:, HP - 1:HP, :], 0.0)
                            eng.memset(t[:, :, 0:1], 0.0)
                            eng.memset(t[:, :, HP - 1:HP], 0.0)
                dmaeng = {0: nc.sync, 2: nc.scalar, 1: nc.sync, 3: nc.scalar}
                for rows in ((0, 32), (32, HW)):
                    for b in (0, 2, 1, 3):
                        dmaeng[b].dma_start(
                            out=xt[b][0:C, rows[0] + 1:rows[1] + 1, 1:HW + 1],
                            in_=x_ext[b:b + 1, :, rows[0]:rows[1], :]
                                .transpose([1, 0, 2, 3]))
                        sl = xt[b][0:C, rows[0] + 1:rows[1] + 1, 1:HW + 1]
                        nc.vector.tensor_tensor(
                            out=xt[b][C:2 * C, rows[0] + 1:rows[1] + 1,
                                      1:HW + 1],
                            in0=sl, in1=sl, op=MULT)

            def emit_B1(r):
                """Layer-1 matmuls/evictions/psum-stats + deferred xres DMA."""
                d = dbuf.tile([2 * C, 2, HW, HW], fp16, tag="d")
                st['d'] = d
                ev0 = None
                for p in range(NPAIR):
                    for i in (0, 1):
                        ev = layer_unit(xt, w1, 1, d, sumd1, sumps1, p, i)
                        if ev0 is None:
                            ev0 = ev
                # residual copy of x in the 128-wide pair layout; needed only
                # at the end, so defer past L1 start to keep rings free
                for b in range(B_LOCAL):
                    pb, i = _pb(b)
                    xr = nc.gpsimd.dma_start(
                        out=xres[pb:pb + C, i, :, :],
                        in_=x_ext[b:b + 1, :, :, :].transpose([1, 0, 2, 3]))
                    tile.add_dep_helper(xr.ins, ev0.ins,
                                        reason="defer xres DMA past L1 start")

            def emit_B2(r):
                gstats1 = _stats_allreduce(nc, pool, dram, sumd1, sumps1,
                                           "l1")
                st['aff1'] = _bn_affine(nc, pool, gstats1, cst[:, 0:2], g1,
                                        b1, eps, "l1")

            def glue_chunk(q, b_list):
                """y = relu(s1*d + t1) rows 14q..14q+14 (DVE, 2 ops) + y^2."""
                d = st['d']
                s1, t1 = st['aff1']
                r0 = 14 * q
                for b in b_list:
                    pb, i = _pb(b)
                    ysl = yt[b][0:C, r0 + 1:r0 + 15, 1:HW + 1]
                    nc.vector.tensor_scalar(
                        out=ysl, in0=d[pb:pb + C, i, r0:r0 + 14, :],
                        scalar1=s1[pb:pb + C, 0:1], scalar2=t1[pb:pb + C, 0:1],
                        op0=MULT, op1=ADD)
                    nc.vector.tensor_scalar(
                        out=ysl, in0=ysl, scalar1=0.0, scalar2=None, op0=MAX)
                    nc.vector.tensor_tensor(
                        out=yt[b][C:2 * C, r0 + 1:r0 + 15, 1:HW + 1],
                        in0=ysl, in1=ysl, op=MULT)

            def emit_final(fin):
                """out = relu(s2*d + t2 + x): 2 DVE ops per chunk; DMA on
                sync/scalar rings (gpsimd stays free for collectives)."""
                d, s2, t2 = fin
                rings = [nc.sync, nc.scalar]
                ri = 0
                for i in range(2):
                    for q in range(4):
                        rs = slice(14 * q, 14 * q + 14)
                        nc.vector.scalar_tensor_tensor(
                            out=afin[:, i, rs, :], in0=d[:, i, rs, :],
                            scalar=s2[:, 0:1], in1=xres[:, i, rs, :],
                            op0=MULT, op1=ADD)
                        nc.vector.tensor_scalar(
                            out=dout[:, i, rs, :], in0=afin[:, i, rs, :],
                            scalar1=t2[:, 0:1], scalar2=0.0,
                            op0=ADD, op1=MAX)
                        for half in range(2):
                            b = 2 * half + i
                            rings[ri % len(rings)].dma_start(
                                out=out_ext[b:b + 1, :, rs, :].transpose(
                                    [1, 0, 2, 3]),
                                in_=dout[64 * half:64 * half + C, i, rs, :])
                            ri += 1

            def emit_C1(r, fin=None):
                """Glue + layer-2 matmuls/evictions/psum-stats. fin, if set,
                is the previous rep's (d, s2, t2): its final phase is emitted
                after the last glue chunk so it executes during this rep's L2
                without FIFO-blocking the glue or the psum-release reduces."""
                d = st['d']
                glue_chunk(0, (0, 2))
                glue_chunk(0, (1, 3))
                glue_chunk(1, (0, 2))
                glue_chunk(1, (1, 3))
                for p in range(NPAIR):
                    if p + 2 <= 3:
                        glue_chunk(p + 2, (0, 2))
                        glue_chunk(p + 2, (1, 3))
                        if p + 2 == 3 and fin is not None:
                            emit_final(fin)
                    for i in (0, 1):
                        layer_unit(yt, w2, 3, d, sumd2, sumps2, p, i)

            def emit_C2(r):
                gstats2 = _stats_allreduce(nc, pool, dram, sumd2, sumps2, "l2")
                st['aff2'] = _bn_affine(nc, pool, gstats2, cst[:, 2:4], g2,
                                        b2, eps, "l2")

            # ---- software-pipelined emission across reps ----
            # Per-engine FIFO order is program order, so rep r's barrier-2 /
            # final work is emitted AFTER rep r+1's L1 (evictions + reduces)
            # to keep PSUM recycling unblocked while collectives are in
            # flight; rep r's final phase rides inside rep r+1's C1.
            emit_A(0)
            emit_B1(0)
            emit_B2(0)
            fin = None
            for r in range(reps):
                emit_C1(r, fin)
                d_r = st['d']
                if r + 1 < reps:
                    emit_A(r + 1)
                    emit_B1(r + 1)
                emit_C2(r)
                fin = (d_r, *st['aff2'])
                if r + 1 < reps:
                    emit_B2(r + 1)
            emit_final(fin)
    nc.compile()
    return nc


_NC_CACHE = None


def _get_nc():
    global _NC_CACHE
    if _NC_CACHE is None:
        _NC_CACHE = build()
    return _NC_CACHE


def _make_in_maps(x, centers1, gamma1, beta1, centers2, gamma2, beta2):
    from ml_dtypes import bfloat16

    def prep_w(centers):
        w = np.empty((2 * C, 9, C), np.float32)
        # centers: [o, d] with d = c*9 + k  ->  w[c, k, o] = -2*centers[o, 9c+k]
        w[:C] = -2.0 * np.ascontiguousarray(
            centers.reshape(C, C, 9).transpose(1, 2, 0))
        w[C:] = 1.0
        return w.astype(bfloat16)

    c1 = np.asarray(centers1, np.float32)
    c2 = np.asarray(centers2, np.float32)
    zero = np.zeros((C,), np.float32)
    # [zero, c2a, zero, c2b, g1, b1, g2, b2] so BN affine can fuse
    # [mu, E[d^2]] into one scalar_tensor_tensor against [zero, c2]
    cst = np.stack([
        zero, (c1 ** 2).sum(1), zero, (c2 ** 2).sum(1),
        np.asarray(gamma1, np.float32), np.asarray(beta1, np.float32),
        np.asarray(gamma2, np.float32), np.asarray(beta2, np.float32),
    ], axis=1).astype(np.float32)
    cst = np.ascontiguousarray(np.tile(cst, (2, 1)))   # duplicate both halves
    common = {
        "w1": prep_w(c1),
        "w2": prep_w(c2),
        "cst": cst,
    }
    xb = np.asarray(x, np.float32).astype(bfloat16)
    in_maps = []
    for c in range(N_CORES):
        m = dict(common)
        sl = slice(c * B_LOCAL, (c + 1) * B_LOCAL)
        m["x"] = np.ascontiguousarray(xb[sl])
        in_maps.append(m)
    return in_maps


def _run(inputs, trace=False, **kw):
    nc = _get_nc()
    in_maps = _make_in_maps(**inputs)
    res = run_bass_kernel_spmd(nc, in_maps, core_ids=list(range(N_CORES)),
                               trace=trace, **kw)
    out = np.concatenate([res.results[c]["out"] for c in range(N_CORES)], axis=0)
    return out.astype(np.float32), res


def kernel(**inputs):
    out, _ = _run(inputs)
    return out


# revision 12
# speedup vs baseline: 2.7604x; 2.7604x over previous
"""Trainium2 Bass kernel for nn_BasicBlock (distance-transform conv BasicBlock).

Computes: relu(bn2(dt_conv2(relu(bn1(dt_conv1(x))))) + x)
where dt_conv is a 3x3 "distance transform conv":
    d[b,o,h,w] = sqrt(||p - c_o||^2),  p = 3x3 zero-padded patch (dim 576)

Strategy (8 NeuronCores, data-parallel over batch 32 -> 4 images/core):
- ||p||^2 - 2 p.c in ONE matmul accumulation group per pixel tile: SBUF
  partitions 0:64 hold x (weights = -2*centers), partitions 64:128 hold x^2
  (weights = 1.0). 9 shifted matmuls (3x3 offsets) accumulate in PSUM,
  K=128, M=64 out channels, N=392 (7 rows x 56), 8 row-groups per image.
- Images (0,2)/(1,3) write psum[0:64] / psum[64:128]. NOTE: the PE runs
  these SERIALLY — explicit tile_position col-tiling was measured at
  287us vs 156us baseline on HW (per-tile LDWEIGHTS serializes against
  the in-flight matmul on the other column half; walrus pins
  --enable-ldw-opt=false so every matmul self-reloads weights). The PE
  stream floor is 2*4*3136*9*2 cycles @ 2.4GHz = 94us/rep; measured
  112us/rep with everything else hidden behind it.
- PSUM tiles span 2 banks (2 row-groups); evictions are one ACT sqrt over
  [128, 2, 7, 56] with accum_out giving per-channel sum(d). sum(d^2) =
  sum(psum) + n*c2 via one DVE XYZ-reduce per bank pair.
- d stored fp16 (quantization ~0.5% of the BN sigma; keeps DVE ops in
  2-byte 4x mode). Glue relu(s1*d+t1) runs on DVE (2 tensor_scalar ops)
  instead of ACT so ACT only does sqrt evictions.
- Sync-BN: fold upper/lower halves, [64,2,2] AllGather across 8 cores +
  local rank-sum, x2 layers. EXACT batch stats are required: Taylor
  mean-from-psum (2.2e-2) and per-core local stats (2.6e-2) both fail
  the 2e-2 gate (numpy-verified; padded-border pixels skew d^2 hard).
- Final: afin = s2*d + xres (DVE STT), out = max(afin + t2, 0) (DVE ts,
  bf16) — pure DVE so rep r+1's ACT evictions are never FIFO-blocked.
- Cross-rep software pipelining (emission order = per-engine FIFO
  order): rep r+1's L1 matmuls/evictions/psum-reduces are emitted
  BEFORE rep r's barrier-2 affine and final phase, so PSUM recycling
  continues while the collective is in flight and the PE only exposes
  barrier-1 (~10us). Rep r's final rides inside rep r+1's C1 after the
  last glue chunk; d is double-buffered to break the cross-rep WAR.

kernel(**inputs) takes FULL unsharded inputs, returns FULL output.
Self-contained: shapes/sharding hardcoded; no file reads.
"""
import numpy as np

from concourse import bacc, mybir, tile
from concourse.bass_utils import run_bass_kernel_spmd

f32 = mybir.dt.float32
bf16 = mybir.dt.bfloat16
fp16 = mybir.dt.float16
ADD = mybir.AluOpType.add
MULT = mybir.AluOpType.mult
SUB = mybir.AluOpType.subtract
MAX = mybir.AluOpType.max
AF = mybir.ActivationFunctionType
XYZ = mybir.AxisListType.XYZ

N_CORES = 8
B_LOCAL = 4            # images per core (32 / 8)
C = 64                 # channels (in == out)
HW = 56                # spatial
HP = HW + 2            # padded
RPG = 7                # rows per matmul group (N = 7*56 = 392)
NGRP = 8               # row-groups per image-pair slot (56 / 7)
NPAIR = 4              # bank pairs (2 groups) per slot
N_GLOBAL = 32 * HW * HW
BN_EPS = 1e-5
TILE_POS = False       # col-tiled image pairs (measured SLOWER on HW: 287us
                       # vs 156us baseline — LDWEIGHTS serializes against the
                       # in-flight matmul on the other column half)


def _pb(b):
    """Partition base and pair-slot index for the 128-wide d layout."""
    return 64 * (b // 2), b % 2


def _bn_affine(nc, pool, gstats, zc2, gamma, beta, eps, name):
    """From [sum(d), sum(d^2)-n*c2] (dup both halves) -> scale s, shift t
    [128,1]. zc2 is a [128, 2] cst slice [zero, c2]."""
    P = 2 * C
    mued = pool.tile([P, 2], f32, tag=f"mued_{name}")
    nvar = pool.tile([P, 1], f32, tag=f"nvar_{name}")
    sd = pool.tile([P, 1], f32, tag=f"sd_{name}")
    inv = pool.tile([P, 1], f32, tag=f"inv_{name}")
    s = pool.tile([P, 1], f32, tag=f"s_{name}")
    st = pool.tile([P, 1], f32, tag=f"st_{name}")
    tt = pool.tile([P, 1], f32, tag=f"t_{name}")
    inv_n = 1.0 / float(N_GLOBAL)
    # [mu, E[d^2]] = gstats * 1/N + [0, c2] in one DVE op
    nc.vector.scalar_tensor_tensor(
        out=mued[:, :], in0=gstats[:, 0:2], scalar=inv_n, in1=zc2,
        op0=MULT, op1=ADD)
    mu, ed2 = mued[:, 0:1], mued[:, 1:2]
    # -var = mu*mu - E[d^2] in one STT; sqrt flips the sign via scale=-1
    nc.vector.scalar_tensor_tensor(
        out=nvar[:, :], in0=mu, scalar=mu, in1=ed2, op0=MULT, op1=SUB)
    nc.scalar.activation(out=sd[:, :], in_=nvar[:, :], func=AF.Sqrt,
                         bias=eps[:, 0:1], scale=-1.0)
    nc.vector.reciprocal(out=inv[:, :], in_=sd[:, :])
    nc.vector.tensor_tensor(out=s[:, :], in0=gamma, in1=inv[:, :], op=MULT)
    nc.vector.tensor_tensor(out=st[:, :], in0=mu, in1=s[:, :], op=MULT)
    nc.vector.tensor_tensor(out=tt[:, :], in0=beta, in1=st[:, :], op=SUB)
    return s, tt


def _stats_allreduce(nc, pool, dram, sumd, sumps, name):
    """Reduce [128, 8] stat columns, fold upper half into lower, AllGather
    [64,2,2] across 8 cores + local rank-sum, return [128,2] duplicated
    global sums."""
    red = pool.tile([2 * C, 2], f32, tag=f"red_{name}")
    gstats = pool.tile([2 * C, 2], f32, tag=f"gstats_{name}")
    nc.vector.tensor_reduce(out=red[:, 0:1], in_=sumd[:, :],
                            axis=mybir.AxisListType.X, op=ADD)
    nc.vector.tensor_reduce(out=red[:, 1:2], in_=sumps[:, :],
                            axis=mybir.AxisListType.X, op=ADD)
    # images (2,3) stats live on the upper partition half; DMA both halves
    # side by side into the collective input (DVE tensor ops can't mix
    # partition bases, DMA can) and fold during the rank-sum reduce.
    cc_in = dram.tile([C, 2, 2], f32, tag=f"ccin_{name}")
    nc.sync.dma_start(out=cc_in[:, :, :].rearrange("c s h -> h c s"),
                      in_=red[:, :])
    cc_out = dram.tile([N_CORES * C, 2, 2], f32, tag=f"ccout_{name}")
    gag = pool.tile([C, N_CORES, 2, 2], f32, tag=f"gag_{name}")
    nc.gpsimd.collective_compute(
        "AllGather", mybir.AluOpType.bypass,
        replica_groups=[list(range(N_CORES))],
        ins=[cc_in.opt()],
        outs=[cc_out.opt()],
    )
    nc.sync.dma_start(
        out=gag[:, :, :, :],
        in_=cc_out[:, :, :].rearrange("(r c) s h -> c r s h", r=N_CORES))
    nc.vector.tensor_reduce(out=gstats[0:C, 0:1], in_=gag[:, :, 0, :],
                            axis=mybir.AxisListType.XY, op=ADD)
    nc.vector.tensor_reduce(out=gstats[0:C, 1:2], in_=gag[:, :, 1, :],
                            axis=mybir.AxisListType.XY, op=ADD)
    nc.vector.tensor_copy(out=gstats[C:2 * C, :], in_=gstats[0:C, :])
    return gstats


def build(reps=1):
    nc = bacc.Bacc("TRN2", target_bir_lowering=False, debug=False,
                   num_devices=N_CORES)
    x_ext = nc.declare_dram_parameter("x", [B_LOCAL, C, HW, HW], bf16,
                                      isOutput=False)
    w1_ext = nc.declare_dram_parameter("w1", [2 * C, 9, C], bf16, isOutput=False)
    w2_ext = nc.declare_dram_parameter("w2", [2 * C, 9, C], bf16, isOutput=False)
    # packed [zero | c2a | zero | c2b | g1 | b1 | g2 | b2], dup on both halves
    cst_ext = nc.declare_dram_parameter("cst", [2 * C, 8], f32, isOutput=False)
    out_ext = nc.declare_dram_parameter("out", [B_LOCAL, C, HW, HW], bf16,
                                        isOutput=True)

    with tile.TileContext(nc) as tc:
        with (
            tc.tile_pool(name="big", bufs=1) as big,
            tc.tile_pool(name="dbuf", bufs=2) as dbuf,
            tc.tile_pool(name="small", bufs=1) as pool,
            tc.tile_pool(name="psum", bufs=4, space="PSUM") as psum,
            tc.tile_pool(name="dram", bufs=1, space="DRAM") as dram,
        ):
            w1 = pool.tile([2 * C, 9, C], bf16, tag="w1")
            w2 = pool.tile([2 * C, 9, C], bf16, tag="w2")
            cst = pool.tile([2 * C, 8], f32, tag="cst")
            g1, b1 = cst[:, 4:5], cst[:, 5:6]
            g2, b2 = cst[:, 6:7], cst[:, 7:8]
            eps = pool.tile([2 * C, 1], f32, tag="eps")
            nc.vector.memset(eps[:, :], BN_EPS)
            # constants via the gpsimd SWDGE ring (SP/ACT rings carry x)
            nc.gpsimd.dma_start(out=w1[:, :, :], in_=w1_ext[:, :, :])
            nc.gpsimd.dma_start(out=cst[:, :], in_=cst_ext[:, :])
            nc.gpsimd.dma_start(out=w2[:, :, :], in_=w2_ext[:, :, :])

            # long-lived tiles (one logical tile, reused every rep; subtile
            # deps order cross-rep WAR/RAW). d rotates through 2 buffers so
            # rep r+1's evictions don't wait on rep r's final-phase reads.
            xt = [big.tile([2 * C, HP, HP], bf16, tag=f"xt{b}",
                           name=f"xt{b}") for b in range(B_LOCAL)]
            yt = [big.tile([2 * C, HP, HP], bf16, tag=f"yt{b}",
                           name=f"yt{b}") for b in range(B_LOCAL)]
            xres = big.tile([2 * C, 2, HW, HW], bf16, tag="xres")
            afin = big.tile([2 * C, 2, HW, HW], fp16, tag="afin")
            dout = big.tile([2 * C, 2, HW, HW], bf16, tag="dout")
            sumd1 = pool.tile([2 * C, 2 * NPAIR], f32, tag="sumd1")
            sumps1 = pool.tile([2 * C, 2 * NPAIR], f32, tag="sumps1")
            sumd2 = pool.tile([2 * C, 2 * NPAIR], f32, tag="sumd2")
            sumps2 = pool.tile([2 * C, 2 * NPAIR], f32, tag="sumps2")

            # per-rep state carried between stage emitters
            st = {}

            def layer_unit(src, w, ci, d, sumd, sumps, p, i):
                """Matmuls + eviction + psum-reduce for bank pair p, slot i.
                src[b] is a [128, HP, HP] bf16 tile (x | x^2)."""
                ps = psum.tile([2 * C, 2, NGRP, 64], f32, tag="ps")
                for j in range(2):
                    r0 = (2 * p + j) * RPG
                    for k in range(9):
                        kh, kw = k // 3, k % 3
                        # images i and i+2 on disjoint 64-col PE groups
                        tp = {"tile_position": (0, 0)} if TILE_POS else {}
                        nc.tensor.matmul(
                            ps[0:C, j, 0:RPG, 0:HW],
                            w[:, k, :],
                            src[i][:, r0 + kh:r0 + kh + RPG, kw:kw + HW],
                            start=(k == 0), stop=(k == 8), **tp,
                        )
                        tp = {"tile_position": (0, 64)} if TILE_POS else {}
                        nc.tensor.matmul(
                            ps[C:2 * C, j, 0:RPG, 0:HW],
                            w[:, k, :],
                            src[i + 2][:, r0 + kh:r0 + kh + RPG, kw:kw + HW],
                            start=(k == 0), stop=(k == 8), **tp,
                        )
                col = i * NPAIR + p
                # per-channel sum(psum) (-> sum(d^2) after +n*c2); emitted
                # first so it runs concurrently with the ACT eviction
                nc.vector.tensor_reduce(
                    out=sumps[:, col:col + 1],
                    in_=ps[:, :, 0:RPG, 0:HW],
                    axis=XYZ, op=ADD)
                # d = sqrt(psum + ||c||^2); accum_out gives per-channel sum(d)
                r0 = 2 * p * RPG
                ev = nc.scalar.activation(
                    out=d[:, i, r0:r0 + 2 * RPG, :].rearrange(
                        "c (a b) w -> c a b w", a=2),
                    in_=ps[:, :, 0:RPG, 0:HW],
                    func=AF.Sqrt, bias=cst[:, ci:ci + 1], scale=1.0,
                    accum_out=sumd[:, col:col + 1])
                return ev

            def emit_A(r):
                """Input DMA + squares into padded xt tiles."""
                if r == 0:
                    # zero the pad borders once (interior-only writes after
                    # this keep them zero). x tiles on DVE, y tiles on Pool.
                    for tiles, eng in ((xt, nc.vector), (yt, nc.gpsimd)):
                        for t in tiles:
                            eng.memset(t[:, 0:1, :], 0.0)
                            eng.memset(t[:, HP - 1:HP, :], 0.0)
                            eng.memset(t[:, :, 0:1], 0.0)
                            eng.memset(t[:, :, HP - 1:HP], 0.0)
                dmaeng = {0: nc.sync, 2: nc.scalar, 1: nc.sync, 3: nc.scalar}
                for rows in ((0, 32), (32, HW)):
                    for b in (0, 2, 1, 3):
                        dmaeng[b].dma_start(
                            out=xt[b][0:C, rows[0] + 1:rows[1] + 1, 1:HW + 1],
                            in_=x_ext[b:b + 1, :, rows[0]:rows[1], :]
                                .transpose([1, 0, 2, 3]))
                        sl = xt[b][0:C, rows[0] + 1:rows[1] + 1, 1:HW + 1]
                        nc.vector.tensor_tensor(
                            out=xt[b][C:2 * C, rows[0] + 1:rows[1] + 1,
                                      1:HW + 1],
                            in0=sl, in1=sl, op=MULT)

            def emit_B1(r):
                """Layer-1 matmuls/evictions/psum-stats + deferred xres DMA."""
                d = dbuf.tile([2 * C, 2, HW, HW], fp16, tag="d")
                st['d'] = d
                ev0 = None
                for p in range(NPAIR):
                    for i in (0, 1):
                        ev = layer_unit(xt, w1, 1, d, sumd1, sumps1, p, i)
                        if ev0 is None:
                            ev0 = ev
                # residual copy of x in the 128-wide pair layout; needed only
                # at the end, so defer past L1 start to keep rings free
                for b in range(B_LOCAL):
                    pb, i = _pb(b)
                    xr = nc.gpsimd.dma_start(
                        out=xres[pb:pb + C, i, :, :],
                        in_=x_ext[b:b + 1, :, :, :].transpose([1, 0, 2, 3]))
                    tile.add_dep_helper(xr.ins, ev0.ins,
                                        reason="defer xres DMA past L1 start")

            def emit_B2(r):
                gstats1 = _stats_allreduce(nc, pool, dram, sumd1, sumps1,
                                           "l1")
                st['aff1'] = _bn_affine(nc, pool, gstats1, cst[:, 0:2], g1,
                                        b1, eps, "l1")

            def glue_chunk(q, b_list):
                """y = relu(s1*d + t1) rows 14q..14q+14 (DVE, 2 ops) + y^2."""
                d = st['d']
                s1, t1 = st['aff1']
                r0 = 14 * q
                for b in b_list:
                    pb, i = _pb(b)
                    ysl = yt[b][0:C, r0 + 1:r0 + 15, 1:HW + 1]
                    nc.vector.tensor_scalar(
                        out=ysl, in0=d[pb:pb + C, i, r0:r0 + 14, :],
                        scalar1=s1[pb:pb + C, 0:1], scalar2=t1[pb:pb + C, 0:1],
                        op0=MULT, op1=ADD)
                    nc.vector.tensor_scalar(
                        out=ysl, in0=ysl, scalar1=0.0, scalar2=None, op0=MAX)
                    nc.vector.tensor_tensor(
                        out=yt[b][C:2 * C, r0 + 1:r0 + 15, 1:HW + 1],
                        in0=ysl, in1=ysl, op=MULT)

            def emit_final(fin):
                """out = relu(s2*d + t2 + x): 2 DVE ops per chunk; DMA on
                sync/scalar rings (gpsimd stays free for collectives)."""
                d, s2, t2 = fin
                rings = [nc.sync, nc.scalar]
                ri = 0
                for i in range(2):
                    for q in range(4):
                        rs = slice(14 * q, 14 * q + 14)
                        nc.vector.scalar_tensor_tensor(
                            out=afin[:, i, rs, :], in0=d[:, i, rs, :],
                            scalar=s2[:, 0:1], in1=xres[:, i, rs, :],
                            op0=MULT, op1=ADD)
                        nc.vector.tensor_scalar(
                            out=dout[:, i, rs, :], in0=afin[:, i, rs, :],
                            scalar1=t2[:, 0:1], scalar2=0.0,
                            op0=ADD, op1=MAX)
                        for half in range(2):
                            b = 2 * half + i
                            rings[ri % len(rings)].dma_start(
                                out=out_ext[b:b + 1, :, rs, :].transpose(
                                    [1, 0, 2, 3]),
                                in_=dout[64 * half:64 * half + C, i, rs, :])
                            ri += 1

            def emit_C1(r, fin=None):
                """Glue + layer-2 matmuls/evictions/psum-stats. fin, if set,
                is the previous rep's (d, s2, t2): its final phase is emitted
                after the last glue chunk so it executes during this rep's L2
                without FIFO-blocking the glue or the psum-release reduces."""
                d = st['d']
                glue_chunk(0, (0, 2))
                glue_chunk(0, (1, 3))
                glue_chunk(1, (0, 2))
                glue_chunk(1, (1, 3))
                for p in range(NPAIR):
                    if p + 2 <= 3:
                        glue_chunk(p + 2, (0, 2))
                        glue_chunk(p + 2, (1, 3))
                        if p + 2 == 3 and fin is not None:
                            emit_final(fin)
                    for i in (0, 1):
                        layer_unit(yt, w2, 3, d, sumd2, sumps2, p, i)

            def emit_C2(r):
                gstats2 = _stats_allreduce(nc, pool, dram, sumd2, sumps2, "l2")
                st['aff2'] = _bn_affine(nc, pool, gstats2, cst[:, 2:4], g2,
                                        b2, eps, "l2")

            # ---- software-pipelined emission across reps ----
            # Per-engine FIFO order is program order, so rep r's barrier-2 /
            # final work is emitted AFTER rep r+1's L1 (evictions + reduces)
            # to keep PSUM recycling unblocked while collectives are in
            # flight; rep r's final phase rides inside rep r+1's C1.
            emit_A(0)
            emit_B1(0)
            emit_B2(0)
            fin = None
            for r in range(reps):
                emit_C1(r, fin)
                d_r = st['d']
                if r + 1 < reps:
                    emit_A(r + 1)
                    emit_B1(r + 1)
                emit_C2(r)
                fin = (d_r, *st['aff2'])
                if r + 1 < reps:
                    emit_B2(r + 1)
            emit_final(fin)
    nc.compile()
    return nc


_NC_CACHE = None


def _get_nc():
    global _NC_CACHE
    if _NC_CACHE is None:
        _NC_CACHE = build()
    return _NC_CACHE


def _make_in_maps(x, centers1, gamma1, beta1, centers2, gamma2, beta2):
    from ml_dtypes import bfloat16

    def prep_w(centers):
        w = np.empty((2 * C, 9, C), np.float32)
        # centers: [o, d] with d = c*9 + k  ->  w[c, k, o] = -2*centers[o, 9c+k]
        w[:C] = -2.0 * np.ascontiguousarray(
            centers.reshape(C, C, 9).transpose(1, 2, 0))
        w[C:] = 1.0
        return w.astype(bfloat16)

    c1 = np.asarray(centers1, np.float32)
    c2 = np.asarray(centers2, np.float32)
    zero = np.zeros((C,), np.float32)
    # [zero, c2a, zero, c2b, g1, b1, g2, b2] so BN affine can fuse
    # [mu, E[d^2]] into one scalar_tensor_tensor against [zero, c2]
    cst = np.stack([
        zero, (c1 ** 2).sum(1), zero, (c2 ** 2).sum(1),
        np.asarray(gamma1, np.float32), np.asarray(beta1, np.float32),
        np.asarray(gamma2, np.float32), np.asarray(beta2, np.float32),
    ], axis=1).astype(np.float32)
    cst = np.ascontiguousarray(np.tile(cst, (2, 1)))   # duplicate both halves
    common = {
        "w1": prep_w(c1),
        "w2": prep_w(c2),
        "cst": cst,
    }
    xb = np.asarray(x, np.float32).astype(bfloat16)
    in_maps = []
    for c in range(N_CORES):
        m = dict(common)
        sl = slice(c * B_LOCAL, (c + 1) * B_LOCAL)
        m["x"] = np.ascontiguousarray(xb[sl])
        in_maps.append(m)
    return in_maps


def _run(inputs, trace=False, **kw):
    nc = _get_nc()
    in_maps = _make_in_maps(**inputs)
    res = run_bass_kernel_spmd(nc, in_maps, core_ids=list(range(N_CORES)),
                               trace=trace, **kw)
    out = np.concatenate([res.results[c]["out"] for c in range(N_CORES)], axis=0)
    return out.astype(np.float32), res


def kernel(**inputs):
    out, _ = _run(inputs)
    return out
